# revision 28
# baseline (speedup 1.0000x reference)
"""Trainium2 Bass kernel for AlignedTriLoss (global hard-mining triplet loss +
DMLI local-stripe shortest-path loss), SPMD over 8 NeuronCores.

Strategy (row-sharded mining, v3 — no collectives, bf16 data path):
  * Host precomputes everything cheap on CPU: bf16 casts of gf/lf, squared
    norms (-0.5*sq as stacked bf16 hi+lo rows), one-hot label matrices
    (moving: [256,N] 1.0; stationary: [256,NA] -L/2), per-stripe local norms,
    and channel-major pre-transposed own-anchor local features.  No
    AllGather / barrier: each core is fully independent.
  * Each core owns N/8 anchor rows and computes, via a fused bf16 TensorE
    matmul chain with an augmented contraction dimension,
        P[i, j] = gf_i . gf_j  - 0.5*sq_j - 0.5*L*[t_i == t_j]
    so that  -2*P[i, j] + sq_i = d2[i, j] + L*eq[i, j].
    Row min/max of P (+ DVE max_index straight on PSUM) give the hardest
    positive / negative values *and* their column indices.  Columns are
    processed in quarters of 1024; within a quarter the 4 anchor tiles are
    split into two halves so the 8 PSUM banks double-buffer: the PE works on
    one half while the DVE drains the other.
  * lf rows (stripes + their norms, one bf16 row each) for the mined indices
    are fetched with indirect DMA, transposed to channel-major with the DMA
    xbar (no PE/DVE involvement), run through 16-anchor-blocked gram matmuls
    with strided operand APs, the block-diagonal 8x8 gram blocks are
    extracted with a strided DMA via a DRAM bounce, normalized with a single
    sqrt+tanh pass per anchor tile on ScalarE, and the 8x8 shortest-path DP
    runs as an anti-diagonal wavefront on VectorE.
Host side only shards/preprocesses inputs, concatenates per-core outputs and
takes means.
"""

import numpy as np
import ml_dtypes

import concourse.bass as bass
import concourse.bacc as bacc
import concourse.mybir as mybir
import concourse.tile as tile
from concourse.masks import make_identity
from concourse.bass_utils import run_bass_kernel_spmd

F32 = mybir.dt.float32
U32 = mybir.dt.uint32
BF16 = mybir.dt.bfloat16
AF = mybir.ActivationFunctionType
ALU = mybir.AluOpType
AX = mybir.AxisListType

P = 128
MARGIN = 0.3
EPS = 1e-12
LBL = 16384.0   # label-match offset; > max d2 (~5k), exact in bf16
BIG = 1e30
BIGI = 65536.0


def _sub(base_ap, off, free_dims):
    """AP at base+off with explicit free dims (keeps base partition dim)."""
    return bass.AP(base_ap.tensor, base_ap.offset + off, [base_ap.ap[0]] + free_dims)


def build_program(N=4096, DG=2048, M=8, C=256, NC=8, NCLS=256):
    NA = N // NC            # anchors per core
    MT = NA // P            # anchor tiles per core
    KG = DG // P            # gf k-tiles
    OH = NCLS // P          # one-hot k-tiles
    CH = C // P             # 128-chunks per local stripe
    G16 = P // (2 * M)      # anchor groups per tile (8 groups of 16)
    NBW = 512               # mining psum tile width
    NQB = 2                 # psum tiles (column blocks) per anchor tile
    QW = NQB * NBW          # columns per quarter
    NQ = N // QW            # quarters
    NPART = NQ * NQB        # per-row partials
    MC = M * C
    MCX = MC + M            # gathered row: stripes + stripe norms
    NK = KG + OH + 1        # mining k rounds (gf, onehot, stacked sq hi/lo)
    SZ = (M + 1) * (M + 1)  # padded DP matrix size
    F = M * M
    GA = 2 * M              # anchors per gram group (16)
    assert G16 * GA == P and C % P == 0 and MT % 2 == 0

    nc = bacc.Bacc("TRN2", target_bir_lowering=False, debug=False, num_devices=NC)

    gf_mov = nc.dram_tensor("gf_mov", [DG, N], BF16, kind="ExternalInput").ap()
    gf_st = nc.dram_tensor("gf_st", [DG, NA], BF16, kind="ExternalInput").ap()
    oh_mov_d = nc.dram_tensor("oh_mov", [NCLS, N], BF16, kind="ExternalInput").ap()
    oh_st_d = nc.dram_tensor("oh_st", [NCLS, NA], BF16, kind="ExternalInput").ap()
    sqhl_d = nc.dram_tensor("sqhl", [2, N], BF16, kind="ExternalInput").ap()
    sq_own_d = nc.dram_tensor("sq_own", [NA], F32, kind="ExternalInput").ap()
    lfx_d = nc.dram_tensor("lfx", [N, MCX], BF16, kind="ExternalInput").ap()
    lftx_d = nc.dram_tensor("lftx", [P, MT * MC], BF16, kind="ExternalInput").ap()
    sqx_d = nc.dram_tensor("sqx", [NA, M], F32, kind="ExternalInput").ap()
    out_d = nc.dram_tensor("out", [NA, 8], F32, kind="ExternalOutput").ap()

    with tile.TileContext(nc) as tc:
        cst = tc.alloc_tile_pool(name="cst", bufs=1)
        mine_p = tc.alloc_tile_pool(name="mine", bufs=1)

        # ---- stationary mining operands (lead the scalar DGE queue) ----
        st_p0 = tc.alloc_tile_pool(name="st", bufs=KG)
        st = []
        for k in range(KG):
            t = st_p0.tile([P, NA], BF16, tag="st", name=f"st{k}")
            nc.scalar.dma_start(t[:], gf_st[k * P:(k + 1) * P, :])
            st.append(t)
        oh_st = []
        for h in range(OH):
            t = cst.tile([P, NA], BF16, name=f"ohst{h}", tag=f"ohst{h}")
            nc.scalar.dma_start(t[:], oh_st_d[h * P:(h + 1) * P, :])
            oh_st.append(t)
        oh_mov = []
        for h in range(OH):
            t = cst.tile([P, N], BF16, name=f"ohm{h}", tag=f"ohm{h}")
            nc.gpsimd.dma_start(t[:], oh_mov_d[h * P:(h + 1) * P, :])
            oh_mov.append(t)
        sqh = cst.tile([2, N], BF16, name="sqh", tag="sqh")
        nc.gpsimd.dma_start(sqh[:], sqhl_d[:])
        ones2 = cst.tile([2, P], BF16)
        nc.gpsimd.memset(ones2[:], 1.0)
        sqo, sqx, txs = [], [], []
        for mt in range(MT):
            s = cst.tile([P, 1], F32, name=f"sqo{mt}", tag=f"sqo{mt}")
            nc.scalar.dma_start(s[:], sq_own_d[mt * P:(mt + 1) * P])
            sqo.append(s)
            s = cst.tile([P, M], F32, name=f"sqx{mt}", tag=f"sqx{mt}")
            nc.scalar.dma_start(s[:], sqx_d[mt * P:(mt + 1) * P, :])
            sqx.append(s)
            t = cst.tile([P, MC], BF16, name=f"tx{mt}", tag=f"tx{mt}")
            nc.scalar.dma_start(t[:], lftx_d[:, mt * MC:(mt + 1) * MC])
            txs.append(t)

        pvq = [mine_p.tile([P, 2 * NPART], F32, tag=f"pvq{mt}", name=f"pvq{mt}")
               for mt in range(MT)]
        piq = [mine_p.tile([P, 2 * NPART], F32, tag=f"piq{mt}", name=f"piq{mt}")
               for mt in range(MT)]

        # ---- mining matmul ----
        with tc.tile_pool(name="minps", bufs=1, space="PSUM") as mmps, \
                tc.tile_pool(name="mov", bufs=KG + 2) as mov_p, \
                tc.tile_pool(name="mq", bufs=4) as mq_p:
            for q in range(NQ):
                qs = q * QW
                movf = []
                for k in range(KG):
                    t = mov_p.tile([P, QW], BF16, tag="movf", name=f"movf{k}")
                    nc.sync.dma_start(t[:], gf_mov[k * P:(k + 1) * P, qs:qs + QW])
                    movf.append(t)
                for half in range(2):
                    mts = (0, 1) if half == 0 else (2, 3)
                    pt = {}
                    for k in range(NK):
                        for mt in mts:
                            if k < KG:
                                lh = st[k][:, mt * P:(mt + 1) * P]
                                rhf = movf[k][:]
                            elif k < KG + OH:
                                lh = oh_st[k - KG][:, mt * P:(mt + 1) * P]
                                rhf = oh_mov[k - KG][:, qs:qs + QW]
                            else:
                                lh = ones2[:]
                                rhf = sqh[:, qs:qs + QW]
                            for nb in range(NQB):
                                if k == 0:
                                    pt[(mt, nb)] = mmps.tile(
                                        [P, NBW], F32, tag=f"mps{mt}_{nb}",
                                        name=f"mps{mt}_{nb}")
                                rh = rhf[:, nb * NBW:(nb + 1) * NBW]
                                nc.tensor.matmul(
                                    pt[(mt, nb)][:], lh, rh,
                                    start=(k == 0), stop=(k == NK - 1))
                    # drain this half's PSUM (overlaps other half's matmuls)
                    for mt in mts:
                        for nb in range(NQB):
                            ix = q * NQB + nb
                            ps = pt[(mt, nb)][:]
                            nc.vector.tensor_reduce(
                                pvq[mt][:, ix:ix + 1], ps, axis=AX.X, op=ALU.min)
                            nc.vector.tensor_reduce(
                                pvq[mt][:, NPART + ix:NPART + ix + 1], ps,
                                axis=AX.X, op=ALU.max)
                            i8a = mq_p.tile([P, 8], U32, tag="i8a", name=f"i8a{mt}")
                            i8b = mq_p.tile([P, 8], U32, tag="i8b", name=f"i8b{mt}")
                            nc.vector.max_index(
                                i8a[:], pvq[mt][:, ix:ix + 1].to_broadcast([P, 8]),
                                ps)
                            nc.vector.max_index(
                                i8b[:],
                                pvq[mt][:, NPART + ix:NPART + ix + 1]
                                .to_broadcast([P, 8]), ps)
                            off = float(qs + nb * NBW)
                            nc.gpsimd.tensor_copy(
                                piq[mt][:, ix:ix + 1], i8a[:, 0:1])
                            nc.gpsimd.tensor_copy(
                                piq[mt][:, NPART + ix:NPART + ix + 1], i8b[:, 0:1])
                            if off:
                                nc.gpsimd.tensor_scalar(
                                    piq[mt][:, ix:ix + 1], piq[mt][:, ix:ix + 1],
                                    off, None, op0=ALU.add)
                                nc.gpsimd.tensor_scalar(
                                    piq[mt][:, NPART + ix:NPART + ix + 1],
                                    piq[mt][:, NPART + ix:NPART + ix + 1],
                                    off, None, op0=ALU.add)

        # ---- combine partials: values, first-occurrence indices, loss ----
        out_tiles, apx_t, anx_t = [], [], []
        for mt in range(MT):
            ot = mine_p.tile([P, 8], F32, name=f"ot{mt}", tag=f"ot{mt}")
            out_tiles.append(ot)
            mn = mine_p.tile([P, 1], F32, tag=f"mn{mt}")
            mx = mine_p.tile([P, 1], F32, tag=f"mx{mt}")
            nc.vector.tensor_reduce(mn[:], pvq[mt][:, 0:NPART], axis=AX.X,
                                    op=ALU.min)
            nc.vector.tensor_reduce(mx[:], pvq[mt][:, NPART:2 * NPART],
                                    axis=AX.X, op=ALU.max)
            idxs = []
            for side, vref in ((0, mn), (1, mx)):
                msk = mine_p.tile([P, NPART], F32, tag=f"msk{mt}_{side}")
                nc.vector.tensor_tensor(
                    msk[:], pvq[mt][:, side * NPART:(side + 1) * NPART],
                    vref[:, 0:1].to_broadcast([P, NPART]), op=ALU.is_equal)
                cnd = mine_p.tile([P, NPART], F32, tag=f"cnd{mt}_{side}")
                nc.vector.tensor_scalar(
                    cnd[:], piq[mt][:, side * NPART:(side + 1) * NPART],
                    -BIGI, None, op0=ALU.add)
                nc.vector.tensor_tensor(cnd[:], cnd[:], msk[:], op=ALU.mult)
                nc.vector.tensor_scalar(cnd[:], cnd[:], BIGI, None, op0=ALU.add)
                sel = mine_p.tile([P, 1], F32, tag=f"sel{mt}_{side}")
                nc.vector.tensor_reduce(sel[:], cnd[:], axis=AX.X, op=ALU.min)
                iu = mine_p.tile([P, 1], U32, tag=f"iu{mt}_{side}")
                nc.vector.tensor_copy(iu[:], sel[:])
                nc.gpsimd.tensor_copy(ot[:, 4 + side:5 + side], sel[:])
                idxs.append(iu)
            apx_t.append(idxs[0])
            anx_t.append(idxs[1])
            # d2_ap = -2*mn - L + sq_i ; d2_an = -2*mx + sq_i
            t1 = mine_p.tile([P, 2], F32, tag=f"t1{mt}")
            nc.vector.tensor_scalar(t1[:, 0:1], mn[:], -2.0, -LBL,
                                    op0=ALU.mult, op1=ALU.add)
            nc.vector.tensor_scalar(t1[:, 1:2], mx[:], -2.0, None, op0=ALU.mult)
            d2 = mine_p.tile([P, 2], F32, tag=f"d2{mt}")
            nc.vector.tensor_tensor(
                d2[:], t1[:], sqo[mt][:, 0:1].to_broadcast([P, 2]), op=ALU.add)
            nc.vector.tensor_scalar(d2[:], d2[:], EPS, None, op0=ALU.max)
            nc.scalar.activation(ot[:, 2:4], d2[:], AF.Sqrt)
            gsub = mine_p.tile([P, 1], F32, tag=f"gs{mt}")
            nc.vector.tensor_tensor(gsub[:], ot[:, 2:3], ot[:, 3:4],
                                    op=ALU.subtract)
            nc.vector.tensor_scalar(ot[:, 0:1], gsub[:], MARGIN, 0.0,
                                    op0=ALU.add, op1=ALU.max)
            nc.gpsimd.tensor_copy(ot[:, 6:7], mn[:])
            nc.gpsimd.tensor_copy(ot[:, 7:8], mx[:])
        st_p0.release()

        # ---- DMLI local loss (phase-batched across anchor tiles) ----
        with tc.tile_pool(name="dml", bufs=1) as dml, \
                tc.tile_pool(name="dmt", bufs=2 * MT) as dmt, \
                tc.tile_pool(name="gps", bufs=4, space="PSUM") as gps, \
                tc.tile_pool(name="gsb", bufs=4) as gsb_p, \
                tc.tile_pool(name="gdr", bufs=4, space="DRAM") as gdr, \
                tc.tile_pool(name="dp", bufs=1) as dpp:
            # phase 1: gathers + stripe-norm extraction
            ys, sqys = [], []
            for mt in range(MT):
                for pn, idx in ((0, apx_t[mt]), (1, anx_t[mt])):
                    y = dml.tile([P, MCX], BF16, tag=f"y{mt}{pn}",
                                 name=f"y{mt}{pn}")
                    nc.gpsimd.indirect_dma_start(
                        out=y[:], out_offset=None, in_=lfx_d[:],
                        in_offset=bass.IndirectOffsetOnAxis(ap=idx[:, 0:1],
                                                            axis=0))
                    sq = dpp.tile([P, M], F32, tag=f"sqy{mt}{pn}",
                                  name=f"sqy{mt}{pn}")
                    nc.gpsimd.tensor_copy(sq[:], y[:, MC:MC + M])
                    ys.append(y)
                    sqys.append(sq)

            # phase 2: xbar DMA transposes to channel-major staging
            # (tty col = (b*CH + h)*P + anchor), then interleave copies on
            # Pool/DVE into ty col = h*M*P + anchor*M + b so each gram (g, h)
            # operand is one contiguous 128-col slice
            tys = []
            for t_i, y in enumerate(ys):
                tty = dmt.tile([P, MC], BF16, tag="tty", name=f"tty{t_i}")
                eng = (nc.sync, nc.scalar)[t_i % 2]
                eng.dma_start_transpose(
                    tty[:].rearrange("p (ch c) -> p ch c", ch=M * CH),
                    y[:, 0:MC])
                ty = dmt.tile([P, MC], BF16, tag="ty", name=f"ty{t_i}")
                for ch in range(M * CH):
                    b_, h_ = divmod(ch, CH)
                    eng = (nc.gpsimd, nc.vector)[(t_i + ch) % 2]
                    eng.tensor_copy(
                        _sub(ty[:], h_ * M * P + b_, [[M, P]]),
                        tty[:, ch * P:(ch + 1) * P])
                tys.append(ty)

            # phase 3: grams (strided operands) + diag extraction via bounce
            dts = []
            ext_engines = (nc.sync, nc.scalar, nc.gpsimd)
            for t_i, ty in enumerate(tys):
                mt = t_i // 2
                tx = txs[mt]
                gsb = gsb_p.tile([P, G16 * P], BF16, tag="gsb",
                                 name=f"gsb{t_i}")
                for g in range(G16):
                    gp = gps.tile([P, P], F32, tag="gram", name=f"gram{g}")
                    for h in range(CH):
                        off = h * M * P + g * P
                        nc.tensor.matmul(gp[:], tx[:, off:off + P],
                                         ty[:, off:off + P],
                                         start=(h == 0), stop=(h == CH - 1))
                    nc.vector.tensor_copy(gsb[:, g * P:(g + 1) * P], gp[:])
                gd = gdr.tile([P * G16 * P], BF16, tag="gd", name=f"gd{t_i}")
                nc.sync.dma_start(gd[:], gsb[:])
                dt_ = dpp.tile([P, F], BF16, tag=f"dl{t_i}", name=f"dl{t_i}")
                for g in range(G16):
                    # gp[(j*M + a), (j'*M + b)]; gd flat addr of diag element
                    # (j, a, b) of group g: (j*M + a)*(G16*P) + g*P + j*M + b
                    ext_in = bass.AP(gd[:].tensor, gd[:].offset + g * P,
                                     [[M * G16 * P + M, GA],
                                      [G16 * P, M], [1, M]])
                    # dt_ col = a*M + b
                    ext_out = bass.AP(dt_[:].tensor, g * GA * F,
                                      [[F, GA], [M, M], [1, M]])
                    ext_engines[g % 3].dma_start(ext_out, ext_in)
                dts.append(dt_)

            # phase 4: d2 assembly + single sqrt/tanh pass per anchor tile
            dists, u2s = [], []
            for mt in range(MT):
                u2 = dpp.tile([P, 2 * F], F32, tag=f"u2{mt}", name=f"u2{mt}")
                for pn in range(2):
                    dd = dts[2 * mt + pn]
                    sqy = sqys[2 * mt + pn]
                    us = u2[:, pn * F:(pn + 1) * F]
                    # u2 col = pn*F + a*M + b
                    nc.vector.tensor_scalar(
                        us, _sub(dd[:], 0, [[M, M], [1, M]]), -2.0, None,
                        op0=ALU.mult)
                    nc.vector.tensor_tensor(
                        us, us, _sub(sqy[:], 0, [[0, M], [1, M]]), op=ALU.add)
                    nc.vector.tensor_tensor(
                        us, us, _sub(sqx[mt][:], 0, [[1, M], [0, M]]),
                        op=ALU.add)
                nc.vector.tensor_scalar(u2[:], u2[:], EPS, None, op0=ALU.max)
                dist = dpp.tile([P, 2 * SZ], F32, tag=f"dist{mt}",
                                name=f"dist{mt}")
                nc.gpsimd.memset(dist[:], BIG)
                nc.gpsimd.memset(dist[:, 1:2], 0.0)
                nc.gpsimd.memset(dist[:, SZ + 1:SZ + 2], 0.0)
                nc.scalar.activation(u2[:], u2[:], AF.Sqrt)
                dists.append(dist)
                u2s.append(u2)
            for mt in range(MT):
                for pn in range(2):
                    nc.scalar.activation(
                        _sub(dists[mt][:], pn * SZ + (M + 1) + 1,
                             [[M + 1, M], [1, M]]),
                        _sub(u2s[mt][:], pn * F, [[M, M], [1, M]]),
                        AF.Tanh, scale=0.5)

            # phase 5: DP wavefront + margin relu + store
            for mt in range(MT):
                dist = dists[mt]
                tmp = dpp.tile([P, 2 * M], F32, tag=f"dptmp{mt}")
                for k in range(2, 2 * M + 1):
                    a_lo = max(1, k - M)
                    n = min(M, k - 1) - a_lo + 1
                    s1 = M * a_lo + k - (M + 1)
                    s2 = M * a_lo + k - 1
                    dst = M * a_lo + k
                    nc.vector.tensor_tensor(
                        _sub(tmp[:], 0, [[M, 2], [1, n]]),
                        _sub(dist[:], s1, [[SZ, 2], [M, n]]),
                        _sub(dist[:], s2, [[SZ, 2], [M, n]]), op=ALU.min)
                    nc.vector.tensor_tensor(
                        _sub(dist[:], dst, [[SZ, 2], [M, n]]),
                        _sub(tmp[:], 0, [[M, 2], [1, n]]),
                        _sub(dist[:], dst, [[SZ, 2], [M, n]]), op=ALU.add)
                fin = M * (M + 2)
                lsub = dpp.tile([P, 1], F32, tag=f"lsub{mt}")
                nc.vector.tensor_tensor(
                    lsub[:], dist[:, fin:fin + 1],
                    dist[:, SZ + fin:SZ + fin + 1], op=ALU.subtract)
                nc.vector.tensor_scalar(out_tiles[mt][:, 1:2], lsub[:],
                                        MARGIN, 0.0, op0=ALU.add, op1=ALU.max)
                nc.sync.dma_start(out_d[mt * P:(mt + 1) * P, :],
                                  out_tiles[mt][:])
        mine_p.release()
        cst.release()

    nc.compile()
    return nc


_CACHE = {}


def _get_program(cfg):
    if cfg not in _CACHE:
        _CACHE[cfg] = build_program(*cfg)
    return _CACHE[cfg]


def make_in_maps(gf, lf, targets, NC, NCLS=256):
    N, DG = gf.shape
    M, C = lf.shape[1], lf.shape[2]
    NA = N // NC
    MT = NA // P
    CH = C // P
    MC = M * C
    BF = ml_dtypes.bfloat16

    gf = np.asarray(gf, dtype=np.float32)
    t = np.asarray(targets).astype(np.int64)
    gfb = gf.astype(BF)
    gf_mov = np.ascontiguousarray(gfb.T)
    sq = np.einsum('nd,nd->n', gf, gf, dtype=np.float64).astype(np.float32)
    msq = -0.5 * sq
    hi = msq.astype(BF)
    lo = (msq - hi.astype(np.float32)).astype(BF)
    sqhl = np.ascontiguousarray(np.stack([hi, lo]))
    oh_mov = np.zeros((NCLS, N), dtype=BF)
    oh_mov[t, np.arange(N)] = 1.0
    lf32 = np.asarray(lf, dtype=np.float32)
    lfb = lf32.astype(BF)
    sql = np.einsum('nmc,nmc->nm', lf32, lf32).astype(np.float32)
    lfx = np.ascontiguousarray(
        np.concatenate([lfb.reshape(N, MC), sql.astype(BF)], axis=1))

    maps = []
    for c in range(NC):
        sl = slice(c * NA, (c + 1) * NA)
        oh_st = np.zeros((NCLS, NA), dtype=BF)
        oh_st[t[sl], np.arange(NA)] = -0.5 * LBL
        # tx layout: lftx[p, mt*MC + h*M*P + anchor*M + a]
        #          = lf[c*NA + mt*P + anchor, a, h*P + p]
        arr = lfb[sl].reshape(MT, P, M, CH, P)        # [mt, anchor, a, h, p]
        lftx = np.ascontiguousarray(
            arr.transpose(4, 0, 3, 1, 2).reshape(P, MT * MC))
        maps.append({
            "gf_mov": gf_mov,
            "gf_st": np.ascontiguousarray(gfb[sl].T),
            "oh_mov": oh_mov,
            "oh_st": oh_st,
            "sqhl": sqhl,
            "sq_own": np.ascontiguousarray(sq[sl]),
            "lfx": lfx,
            "lftx": lftx,
            "sqx": np.ascontiguousarray(sql[sl]),
        })
    return maps


def kernel(gf, lf, targets):
    NC = 8
    N, DG = gf.shape
    M, C = lf.shape[1], lf.shape[2]
    nc = _get_program((N, DG, M, C, NC, 256))
    in_maps = make_in_maps(gf, lf, targets, NC)
    res = run_bass_kernel_spmd(nc, in_maps, core_ids=list(range(NC)))
    outs = np.concatenate([res.results[c]["out"] for c in range(NC)], axis=0)
    g = outs[:, 0].mean(dtype=np.float64)
    l = outs[:, 1].mean(dtype=np.float64)
    return np.array([g, l], dtype=np.float32)


# revision 31
# speedup vs baseline: 1.0975x; 1.0975x over previous
"""Trainium2 Bass kernel for AlignedTriLoss (global hard-mining triplet loss +
DMLI local-stripe shortest-path loss), SPMD over 8 NeuronCores.

Strategy (row-sharded mining, v3 — no collectives, bf16 data path):
  * Host precomputes everything cheap on CPU: bf16 casts of gf/lf, squared
    norms (-0.5*sq as stacked bf16 hi+lo rows), one-hot label matrices
    (moving: [256,N] 1.0; stationary: [256,NA] -L/2), per-stripe local norms,
    and channel-major pre-transposed own-anchor local features.  No
    AllGather / barrier: each core is fully independent.
  * Each core owns N/8 anchor rows and computes, via a fused bf16 TensorE
    matmul chain with an augmented contraction dimension,
        P[i, j] = gf_i . gf_j  - 0.5*sq_j - 0.5*L*[t_i == t_j]
    so that  -2*P[i, j] + sq_i = d2[i, j] + L*eq[i, j].
    Row min/max of P (+ DVE max_index straight on PSUM) give the hardest
    positive / negative values *and* their column indices.  Columns are
    processed in quarters of 1024; within a quarter the 4 anchor tiles are
    split into two halves so the 8 PSUM banks double-buffer: the PE works on
    one half while the DVE drains the other.
  * lf rows (stripes + their norms, one bf16 row each) for the mined indices
    are fetched with indirect DMA, transposed to channel-major with the DMA
    xbar (no PE/DVE involvement), run through 16-anchor-blocked gram matmuls
    with strided operand APs, the block-diagonal 8x8 gram blocks are
    extracted with a strided DMA via a DRAM bounce, normalized with a single
    sqrt+tanh pass per anchor tile on ScalarE, and the 8x8 shortest-path DP
    runs as an anti-diagonal wavefront on VectorE.
Host side only shards/preprocesses inputs, concatenates per-core outputs and
takes means.
"""

import numpy as np
import ml_dtypes

import concourse.bass as bass
import concourse.bacc as bacc
import concourse.mybir as mybir
import concourse.tile as tile
from concourse.masks import make_identity
from concourse.bass_utils import run_bass_kernel_spmd

F32 = mybir.dt.float32
U32 = mybir.dt.uint32
BF16 = mybir.dt.bfloat16
AF = mybir.ActivationFunctionType
ALU = mybir.AluOpType
AX = mybir.AxisListType

P = 128
MARGIN = 0.3
EPS = 1e-12
LBL = 16384.0   # label-match offset; > max d2 (~5k), exact in bf16
BIG = 1e30
BIGI = 65536.0


def _sub(base_ap, off, free_dims):
    """AP at base+off with explicit free dims (keeps base partition dim)."""
    return bass.AP(base_ap.tensor, base_ap.offset + off, [base_ap.ap[0]] + free_dims)


def build_program(N=4096, DG=2048, M=8, C=256, NC=8, NCLS=256):
    NA = N // NC            # anchors per core
    MT = NA // P            # anchor tiles per core
    KG = DG // P            # gf k-tiles
    OH = NCLS // P          # one-hot k-tiles
    CH = C // P             # 128-chunks per local stripe
    G16 = P // (2 * M)      # anchor groups per tile (8 groups of 16)
    NBW = 512               # mining psum tile width
    NQB = 2                 # psum tiles (column blocks) per anchor tile
    QW = NQB * NBW          # columns per quarter
    NQ = N // QW            # quarters
    NPART = NQ * NQB        # per-row partials
    MC = M * C
    MCX = MC + M            # gathered row: stripes + stripe norms
    NK = KG + OH + 1        # mining k rounds (gf, onehot, stacked sq hi/lo)
    SZ = (M + 1) * (M + 1)  # padded DP matrix size
    F = M * M
    GA = 2 * M              # anchors per gram group (16)
    assert G16 * GA == P and C % P == 0 and MT % 2 == 0

    nc = bacc.Bacc("TRN2", target_bir_lowering=False, debug=False, num_devices=NC)

    gf_mov = nc.dram_tensor("gf_mov", [DG, N], BF16, kind="ExternalInput").ap()
    gf_st = nc.dram_tensor("gf_st", [DG, NA], BF16, kind="ExternalInput").ap()
    oh_mov_d = nc.dram_tensor("oh_mov", [NCLS, N], BF16, kind="ExternalInput").ap()
    oh_st_d = nc.dram_tensor("oh_st", [NCLS, NA], BF16, kind="ExternalInput").ap()
    sqhl_d = nc.dram_tensor("sqhl", [2, N], BF16, kind="ExternalInput").ap()
    sq_own_d = nc.dram_tensor("sq_own", [NA], F32, kind="ExternalInput").ap()
    lfx_d = nc.dram_tensor("lfx", [N, MCX], BF16, kind="ExternalInput").ap()
    lftx_d = nc.dram_tensor("lftx", [P, MT * MC], BF16, kind="ExternalInput").ap()
    sqx_d = nc.dram_tensor("sqx", [NA, M], F32, kind="ExternalInput").ap()
    out_d = nc.dram_tensor("out", [NA, 8], F32, kind="ExternalOutput").ap()

    with tile.TileContext(nc) as tc:
        cst = tc.alloc_tile_pool(name="cst", bufs=1)
        mine_p = tc.alloc_tile_pool(name="mine", bufs=1)

        # ---- stationary mining operands (lead the scalar DGE queue) ----
        st_p0 = tc.alloc_tile_pool(name="st", bufs=KG)
        st = []
        for k in range(KG):
            t = st_p0.tile([P, NA], BF16, tag="st", name=f"st{k}")
            nc.scalar.dma_start(t[:], gf_st[k * P:(k + 1) * P, :])
            st.append(t)
        oh_st = []
        for h in range(OH):
            t = cst.tile([P, NA], BF16, name=f"ohst{h}", tag=f"ohst{h}")
            nc.scalar.dma_start(t[:], oh_st_d[h * P:(h + 1) * P, :])
            oh_st.append(t)
        oh_mov = []
        for h in range(OH):
            t = cst.tile([P, N], BF16, name=f"ohm{h}", tag=f"ohm{h}")
            nc.gpsimd.dma_start(t[:], oh_mov_d[h * P:(h + 1) * P, :])
            oh_mov.append(t)
        sqh = cst.tile([2, N], BF16, name="sqh", tag="sqh")
        nc.gpsimd.dma_start(sqh[:], sqhl_d[:])
        ones2 = cst.tile([2, P], BF16)
        nc.gpsimd.memset(ones2[:], 1.0)
        ident_b = cst.tile([P, P], BF16)
        make_identity(nc, ident_b[:])
        sqo, sqx, txs = [], [], []
        for mt in range(MT):
            s = cst.tile([P, 1], F32, name=f"sqo{mt}", tag=f"sqo{mt}")
            nc.scalar.dma_start(s[:], sq_own_d[mt * P:(mt + 1) * P])
            sqo.append(s)
            s = cst.tile([P, M], F32, name=f"sqx{mt}", tag=f"sqx{mt}")
            nc.scalar.dma_start(s[:], sqx_d[mt * P:(mt + 1) * P, :])
            sqx.append(s)
            t = cst.tile([P, MC], BF16, name=f"tx{mt}", tag=f"tx{mt}")
            nc.scalar.dma_start(t[:], lftx_d[:, mt * MC:(mt + 1) * MC])
            txs.append(t)

        pvq = [mine_p.tile([P, 2 * NPART], F32, tag=f"pvq{mt}", name=f"pvq{mt}")
               for mt in range(MT)]
        piq = [mine_p.tile([P, 2 * NPART], F32, tag=f"piq{mt}", name=f"piq{mt}")
               for mt in range(MT)]

        # ---- mining matmul ----
        with tc.tile_pool(name="minps", bufs=1, space="PSUM") as mmps, \
                tc.tile_pool(name="mov", bufs=KG + 2) as mov_p, \
                tc.tile_pool(name="mq", bufs=4) as mq_p:
            for q in range(NQ):
                qs = q * QW
                movf = []
                for k in range(KG):
                    t = mov_p.tile([P, QW], BF16, tag="movf", name=f"movf{k}")
                    nc.sync.dma_start(t[:], gf_mov[k * P:(k + 1) * P, qs:qs + QW])
                    movf.append(t)
                for half in range(2):
                    mts = (0, 1) if half == 0 else (2, 3)
                    pt = {}
                    for k in range(NK):
                        for mt in mts:
                            if k < KG:
                                lh = st[k][:, mt * P:(mt + 1) * P]
                                rhf = movf[k][:]
                            elif k < KG + OH:
                                lh = oh_st[k - KG][:, mt * P:(mt + 1) * P]
                                rhf = oh_mov[k - KG][:, qs:qs + QW]
                            else:
                                lh = ones2[:]
                                rhf = sqh[:, qs:qs + QW]
                            for nb in range(NQB):
                                if k == 0:
                                    pt[(mt, nb)] = mmps.tile(
                                        [P, NBW], F32, tag=f"mps{mt}_{nb}",
                                        name=f"mps{mt}_{nb}")
                                rh = rhf[:, nb * NBW:(nb + 1) * NBW]
                                nc.tensor.matmul(
                                    pt[(mt, nb)][:], lh, rh,
                                    start=(k == 0), stop=(k == NK - 1))
                    # drain this half's PSUM (overlaps other half's matmuls)
                    for mt in mts:
                        for nb in range(NQB):
                            ix = q * NQB + nb
                            ps = pt[(mt, nb)][:]
                            nc.vector.tensor_reduce(
                                pvq[mt][:, ix:ix + 1], ps, axis=AX.X, op=ALU.min)
                            nc.vector.tensor_reduce(
                                pvq[mt][:, NPART + ix:NPART + ix + 1], ps,
                                axis=AX.X, op=ALU.max)
                            i8a = mq_p.tile([P, 8], U32, tag="i8a", name=f"i8a{mt}")
                            i8b = mq_p.tile([P, 8], U32, tag="i8b", name=f"i8b{mt}")
                            nc.vector.max_index(
                                i8a[:], pvq[mt][:, ix:ix + 1].to_broadcast([P, 8]),
                                ps)
                            nc.vector.max_index(
                                i8b[:],
                                pvq[mt][:, NPART + ix:NPART + ix + 1]
                                .to_broadcast([P, 8]), ps)
                            off = float(qs + nb * NBW)
                            nc.gpsimd.tensor_copy(
                                piq[mt][:, ix:ix + 1], i8a[:, 0:1])
                            nc.gpsimd.tensor_copy(
                                piq[mt][:, NPART + ix:NPART + ix + 1], i8b[:, 0:1])
                            if off:
                                nc.gpsimd.tensor_scalar(
                                    piq[mt][:, ix:ix + 1], piq[mt][:, ix:ix + 1],
                                    off, None, op0=ALU.add)
                                nc.gpsimd.tensor_scalar(
                                    piq[mt][:, NPART + ix:NPART + ix + 1],
                                    piq[mt][:, NPART + ix:NPART + ix + 1],
                                    off, None, op0=ALU.add)

        # ---- combine partials: values, first-occurrence indices, loss ----
        out_tiles, apx_t, anx_t = [], [], []
        for mt in range(MT):
            ot = mine_p.tile([P, 8], F32, name=f"ot{mt}", tag=f"ot{mt}")
            out_tiles.append(ot)
            mn = mine_p.tile([P, 1], F32, tag=f"mn{mt}")
            mx = mine_p.tile([P, 1], F32, tag=f"mx{mt}")
            nc.vector.tensor_reduce(mn[:], pvq[mt][:, 0:NPART], axis=AX.X,
                                    op=ALU.min)
            nc.vector.tensor_reduce(mx[:], pvq[mt][:, NPART:2 * NPART],
                                    axis=AX.X, op=ALU.max)
            idxs = []
            for side, vref in ((0, mn), (1, mx)):
                msk = mine_p.tile([P, NPART], F32, tag=f"msk{mt}_{side}")
                nc.vector.tensor_tensor(
                    msk[:], pvq[mt][:, side * NPART:(side + 1) * NPART],
                    vref[:, 0:1].to_broadcast([P, NPART]), op=ALU.is_equal)
                cnd = mine_p.tile([P, NPART], F32, tag=f"cnd{mt}_{side}")
                nc.vector.tensor_scalar(
                    cnd[:], piq[mt][:, side * NPART:(side + 1) * NPART],
                    -BIGI, None, op0=ALU.add)
                nc.vector.tensor_tensor(cnd[:], cnd[:], msk[:], op=ALU.mult)
                nc.vector.tensor_scalar(cnd[:], cnd[:], BIGI, None, op0=ALU.add)
                sel = mine_p.tile([P, 1], F32, tag=f"sel{mt}_{side}")
                nc.vector.tensor_reduce(sel[:], cnd[:], axis=AX.X, op=ALU.min)
                iu = mine_p.tile([P, 1], U32, tag=f"iu{mt}_{side}")
                nc.vector.tensor_copy(iu[:], sel[:])
                nc.gpsimd.tensor_copy(ot[:, 4 + side:5 + side], sel[:])
                idxs.append(iu)
            apx_t.append(idxs[0])
            anx_t.append(idxs[1])
            # d2_ap = -2*mn - L + sq_i ; d2_an = -2*mx + sq_i
            t1 = mine_p.tile([P, 2], F32, tag=f"t1{mt}")
            nc.vector.tensor_scalar(t1[:, 0:1], mn[:], -2.0, -LBL,
                                    op0=ALU.mult, op1=ALU.add)
            nc.vector.tensor_scalar(t1[:, 1:2], mx[:], -2.0, None, op0=ALU.mult)
            d2 = mine_p.tile([P, 2], F32, tag=f"d2{mt}")
            nc.vector.tensor_tensor(
                d2[:], t1[:], sqo[mt][:, 0:1].to_broadcast([P, 2]), op=ALU.add)
            nc.vector.tensor_scalar(d2[:], d2[:], EPS, None, op0=ALU.max)
            nc.scalar.activation(ot[:, 2:4], d2[:], AF.Sqrt)
            gsub = mine_p.tile([P, 1], F32, tag=f"gs{mt}")
            nc.vector.tensor_tensor(gsub[:], ot[:, 2:3], ot[:, 3:4],
                                    op=ALU.subtract)
            nc.vector.tensor_scalar(ot[:, 0:1], gsub[:], MARGIN, 0.0,
                                    op0=ALU.add, op1=ALU.max)
            nc.gpsimd.tensor_copy(ot[:, 6:7], mn[:])
            nc.gpsimd.tensor_copy(ot[:, 7:8], mx[:])
        st_p0.release()

        # ---- DMLI local loss (phase-batched across anchor tiles) ----
        with tc.tile_pool(name="dml", bufs=1) as dml, \
                tc.tile_pool(name="dmt", bufs=4) as dmt, \
                tc.tile_pool(name="tps", bufs=4, space="PSUM") as tps, \
                tc.tile_pool(name="gps", bufs=4, space="PSUM") as gps, \
                tc.tile_pool(name="gsb", bufs=4) as gsb_p, \
                tc.tile_pool(name="gdr", bufs=4, space="DRAM") as gdr, \
                tc.tile_pool(name="dp", bufs=1) as dpp:
            # phase 1: gathers + stripe-norm extraction
            ys, sqys = [], []
            for mt in range(MT):
                for pn, idx in ((0, apx_t[mt]), (1, anx_t[mt])):
                    y = dml.tile([P, MCX], BF16, tag=f"y{mt}{pn}",
                                 name=f"y{mt}{pn}")
                    nc.gpsimd.indirect_dma_start(
                        out=y[:], out_offset=None, in_=lfx_d[:],
                        in_offset=bass.IndirectOffsetOnAxis(ap=idx[:, 0:1],
                                                            axis=0))
                    sq = dpp.tile([P, M], F32, tag=f"sqy{mt}{pn}",
                                  name=f"sqy{mt}{pn}")
                    nc.gpsimd.tensor_copy(sq[:], y[:, MC:MC + M])
                    ys.append(y)
                    sqys.append(sq)

            # phase 2: PE transposes to channel-major; all 8 stripe chunks of
            # one (y, h) land in a single [P, M*P] bf16 PSUM tile, then one
            # strided DVE copy interleaves them into ty col =
            # h*M*P + anchor*M + b so each gram (g, h) operand is one
            # contiguous 128-col slice
            tys = []
            for t_i, y in enumerate(ys):
                ty = dmt.tile([P, MC], BF16, tag="ty", name=f"ty{t_i}")
                for h in range(CH):
                    tp = tps.tile([P, M * P], BF16, tag="tp", name=f"tp{t_i}{h}")
                    for b in range(M):
                        nc.tensor.transpose(
                            tp[:, b * P:(b + 1) * P],
                            y[:, (b * CH + h) * P:(b * CH + h + 1) * P],
                            ident_b[:])
                    nc.vector.tensor_copy(
                        _sub(ty[:], h * M * P, [[1, M], [M, P]]),
                        _sub(tp[:], 0, [[P, M], [1, P]]))
                tys.append(ty)

            # phase 3: grams (strided operands) + diag extraction via bounce
            dts = []
            ext_engines = (nc.sync, nc.scalar, nc.gpsimd)
            for t_i, ty in enumerate(tys):
                mt = t_i // 2
                tx = txs[mt]
                gsb = gsb_p.tile([P, G16 * P], BF16, tag="gsb",
                                 name=f"gsb{t_i}")
                for g in range(G16):
                    gp = gps.tile([P, P], F32, tag="gram", name=f"gram{g}")
                    for h in range(CH):
                        off = h * M * P + g * P
                        nc.tensor.matmul(gp[:], tx[:, off:off + P],
                                         ty[:, off:off + P],
                                         start=(h == 0), stop=(h == CH - 1))
                    nc.vector.tensor_copy(gsb[:, g * P:(g + 1) * P], gp[:])
                gd = gdr.tile([P * G16 * P], BF16, tag="gd", name=f"gd{t_i}")
                nc.sync.dma_start(gd[:], gsb[:])
                dt_ = dpp.tile([P, F], BF16, tag=f"dl{t_i}", name=f"dl{t_i}")
                for g in range(G16):
                    # gp[(j*M + a), (j'*M + b)]; gd flat addr of diag element
                    # (j, a, b) of group g: (j*M + a)*(G16*P) + g*P + j*M + b
                    ext_in = bass.AP(gd[:].tensor, gd[:].offset + g * P,
                                     [[M * G16 * P + M, GA],
                                      [G16 * P, M], [1, M]])
                    # dt_ col = a*M + b
                    ext_out = bass.AP(dt_[:].tensor, g * GA * F,
                                      [[F, GA], [M, M], [1, M]])
                    ext_engines[g % 3].dma_start(ext_out, ext_in)
                dts.append(dt_)

            # phase 4: d2 assembly + single sqrt/tanh pass per anchor tile
            dists, u2s = [], []
            for mt in range(MT):
                u2 = dpp.tile([P, 2 * F], F32, tag=f"u2{mt}", name=f"u2{mt}")
                for pn in range(2):
                    dd = dts[2 * mt + pn]
                    sqy = sqys[2 * mt + pn]
                    us = u2[:, pn * F:(pn + 1) * F]
                    # u2 col = pn*F + a*M + b
                    nc.vector.tensor_scalar(
                        us, _sub(dd[:], 0, [[M, M], [1, M]]), -2.0, None,
                        op0=ALU.mult)
                    nc.vector.tensor_tensor(
                        us, us, _sub(sqy[:], 0, [[0, M], [1, M]]), op=ALU.add)
                    nc.vector.tensor_tensor(
                        us, us, _sub(sqx[mt][:], 0, [[1, M], [0, M]]),
                        op=ALU.add)
                nc.vector.tensor_scalar(u2[:], u2[:], EPS, None, op0=ALU.max)
                dist = dpp.tile([P, 2 * SZ], F32, tag=f"dist{mt}",
                                name=f"dist{mt}")
                nc.gpsimd.memset(dist[:], BIG)
                nc.gpsimd.memset(dist[:, 1:2], 0.0)
                nc.gpsimd.memset(dist[:, SZ + 1:SZ + 2], 0.0)
                nc.scalar.activation(u2[:], u2[:], AF.Sqrt)
                dists.append(dist)
                u2s.append(u2)
            for mt in range(MT):
                for pn in range(2):
                    nc.scalar.activation(
                        _sub(dists[mt][:], pn * SZ + (M + 1) + 1,
                             [[M + 1, M], [1, M]]),
                        _sub(u2s[mt][:], pn * F, [[M, M], [1, M]]),
                        AF.Tanh, scale=0.5)

            # phase 5: DP wavefront + margin relu + store
            for mt in range(MT):
                dist = dists[mt]
                tmp = dpp.tile([P, 2 * M], F32, tag=f"dptmp{mt}")
                for k in range(2, 2 * M + 1):
                    a_lo = max(1, k - M)
                    n = min(M, k - 1) - a_lo + 1
                    s1 = M * a_lo + k - (M + 1)
                    s2 = M * a_lo + k - 1
                    dst = M * a_lo + k
                    nc.vector.tensor_tensor(
                        _sub(tmp[:], 0, [[M, 2], [1, n]]),
                        _sub(dist[:], s1, [[SZ, 2], [M, n]]),
                        _sub(dist[:], s2, [[SZ, 2], [M, n]]), op=ALU.min)
                    nc.vector.tensor_tensor(
                        _sub(dist[:], dst, [[SZ, 2], [M, n]]),
                        _sub(tmp[:], 0, [[M, 2], [1, n]]),
                        _sub(dist[:], dst, [[SZ, 2], [M, n]]), op=ALU.add)
                fin = M * (M + 2)
                lsub = dpp.tile([P, 1], F32, tag=f"lsub{mt}")
                nc.vector.tensor_tensor(
                    lsub[:], dist[:, fin:fin + 1],
                    dist[:, SZ + fin:SZ + fin + 1], op=ALU.subtract)
                nc.vector.tensor_scalar(out_tiles[mt][:, 1:2], lsub[:],
                                        MARGIN, 0.0, op0=ALU.add, op1=ALU.max)
                nc.sync.dma_start(out_d[mt * P:(mt + 1) * P, :],
                                  out_tiles[mt][:])
        mine_p.release()
        cst.release()

    nc.compile()
    return nc


_CACHE = {}


def _get_program(cfg):
    if cfg not in _CACHE:
        _CACHE[cfg] = build_program(*cfg)
    return _CACHE[cfg]


def make_in_maps(gf, lf, targets, NC, NCLS=256):
    N, DG = gf.shape
    M, C = lf.shape[1], lf.shape[2]
    NA = N // NC
    MT = NA // P
    CH = C // P
    MC = M * C
    BF = ml_dtypes.bfloat16

    gf = np.asarray(gf, dtype=np.float32)
    t = np.asarray(targets).astype(np.int64)
    gfb = gf.astype(BF)
    gf_mov = np.ascontiguousarray(gfb.T)
    sq = np.einsum('nd,nd->n', gf, gf, dtype=np.float64).astype(np.float32)
    msq = -0.5 * sq
    hi = msq.astype(BF)
    lo = (msq - hi.astype(np.float32)).astype(BF)
    sqhl = np.ascontiguousarray(np.stack([hi, lo]))
    oh_mov = np.zeros((NCLS, N), dtype=BF)
    oh_mov[t, np.arange(N)] = 1.0
    lf32 = np.asarray(lf, dtype=np.float32)
    lfb = lf32.astype(BF)
    sql = np.einsum('nmc,nmc->nm', lf32, lf32).astype(np.float32)
    lfx = np.ascontiguousarray(
        np.concatenate([lfb.reshape(N, MC), sql.astype(BF)], axis=1))

    maps = []
    for c in range(NC):
        sl = slice(c * NA, (c + 1) * NA)
        oh_st = np.zeros((NCLS, NA), dtype=BF)
        oh_st[t[sl], np.arange(NA)] = -0.5 * LBL
        # tx layout: lftx[p, mt*MC + h*M*P + anchor*M + a]
        #          = lf[c*NA + mt*P + anchor, a, h*P + p]
        arr = lfb[sl].reshape(MT, P, M, CH, P)        # [mt, anchor, a, h, p]
        lftx = np.ascontiguousarray(
            arr.transpose(4, 0, 3, 1, 2).reshape(P, MT * MC))
        maps.append({
            "gf_mov": gf_mov,
            "gf_st": np.ascontiguousarray(gfb[sl].T),
            "oh_mov": oh_mov,
            "oh_st": oh_st,
            "sqhl": sqhl,
            "sq_own": np.ascontiguousarray(sq[sl]),
            "lfx": lfx,
            "lftx": lftx,
            "sqx": np.ascontiguousarray(sql[sl]),
        })
    return maps


def kernel(gf, lf, targets):
    NC = 8
    N, DG = gf.shape
    M, C = lf.shape[1], lf.shape[2]
    nc = _get_program((N, DG, M, C, NC, 256))
    in_maps = make_in_maps(gf, lf, targets, NC)
    res = run_bass_kernel_spmd(nc, in_maps, core_ids=list(range(NC)))
    outs = np.concatenate([res.results[c]["out"] for c in range(NC)], axis=0)
    g = outs[:, 0].mean(dtype=np.float64)
    l = outs[:, 1].mean(dtype=np.float64)
    return np.array([g, l], dtype=np.float32)
